# revision 22
# baseline (speedup 1.0000x reference)
"""Alignment generator (length regulator) on 8 TRN2 NeuronCores.

out[b, f, j] = 1.0  iff  starts[b,j] <= f < ends[b,j]  (ends = cumsum(dur))

Each output row out[b, f, :] is one-hot at token_id[b, f] =
searchsorted(ends[b], f, side='right') (or all-zero when no token covers
frame f). The host computes token_id from the tiny [32, 512] duration input;
each core then generates its 4-row slab of the ~256MB output with one DVE
tensor_scalar(is_equal) per [128, 512] tile (fp32 tensor_scalar runs in the
2x_2P perf mode) and streams it out in ~2MB HWDGE DMAs. The kernel is
write-bandwidth bound: per core ~32MB at the ~358 GB/s HBM-per-core limit
is ~90us, and measured NEFF time is ~94-105us.

Layout: partition p of row b covers the CONTIGUOUS frame span
[p*ntiles, (p+1)*ntiles), so every output DMA is perfectly linear in DRAM
(16KB+ contiguous per partition-descriptor instead of 2KB tile rows).

Raw Bass (no Tile): this walrus build only allows a single sync-wait per
compute/DMA instruction, so all synchronization is explicit standalone
wait_ge with a ring of NBUF buffers and one completion semaphore per buffer
slot (per-slot sems make "slot's previous DMA fully drained" provable from
a 16*m threshold; a single shared sem cannot distinguish engine skew).

Sharding: pure data parallelism, batch dim 32 -> 4 rows per core; no
collectives.
"""

import math
from contextlib import ExitStack

import numpy as np

import concourse.bass as bass
import concourse.mybir as mybir
from concourse.bass_utils import run_bass_kernel_spmd

N_CORES = 8
B = 32          # batch
T = 512         # tokens
P = 128         # SBUF partitions
GROUP = 8       # span steps per output DMA (8*128*512*4B = 2 MiB)
NBUF = 4        # output buffer ring slots

_nc_cache: dict[tuple[int, int], bass.Bass] = {}


def _build(m_frames: int, b_loc: int) -> bass.Bass:
    """Per-core Bass graph writing a [b_loc, m_pad, T] padded output slab."""
    ntiles = math.ceil(m_frames / P)
    m_pad = ntiles * P

    # rounds: (row, first_span_step, n_span_steps). Ramp the first row's
    # chunks (1,1,2,4,...) so the first output DMA is issued as soon as
    # possible after the input lands -- the DMA stream is the bottleneck
    # and every ns it starts earlier is a ns off the kernel.
    rounds = []
    for b in range(b_loc):
        g0 = 0
        for g in [1, 1, 2, 4] if b == 0 else []:
            if g0 + g > ntiles:
                break
            rounds.append((b, g0, g))
            g0 += g
        while g0 < ntiles:
            g = min(GROUP, ntiles - g0)
            rounds.append((b, g0, g))
            g0 += g
    n_rounds = len(rounds)

    nc = bass.Bass()
    # input column layout: [0:T) = iota row J (J[p,j] = j); column
    # (T + b*ntiles + i) = token ids of frames {p*ntiles + i} of row b
    tid = nc.declare_dram_parameter(
        "tid", [P, T + b_loc * ntiles], mybir.dt.float32, isOutput=False
    )
    out = nc.declare_dram_parameter(
        "out", [b_loc, m_pad, T], mybir.dt.float32, isOutput=True
    )

    with ExitStack() as ctx:
        sb = ctx.enter_context(
            nc.sbuf_tensor("sb", [P, T + b_loc * ntiles], mybir.dt.float32)
        )
        bufs = [
            ctx.enter_context(
                nc.sbuf_tensor(f"buf{s}", [P, GROUP * T], mybir.dt.float32)
            )
            for s in range(NBUF)
        ]
        in_sem = ctx.enter_context(nc.semaphore("in_sem"))
        c_sem = ctx.enter_context(nc.semaphore("c_sem"))
        d_sems = [ctx.enter_context(nc.semaphore(f"d_sem{s}")) for s in range(NBUF)]
        block = ctx.enter_context(nc.Block())

        @block.sync
        def _(sync):
            sync.dma_start(out=sb[:, :], in_=tid[:, :]).then_inc(in_sem, 16)
            for r, (b, g0, g) in enumerate(rounds):
                sync.wait_ge(c_sem, r + 1)
                dview = out[b].rearrange("(p i) t -> p (i t)", p=P)[
                    :, g0 * T : (g0 + g) * T
                ]
                sync.dma_start(
                    out=dview, in_=bufs[r % NBUF][:, : g * T]
                ).then_inc(d_sems[r % NBUF], 16)
            # all output bytes landed before the NEFF may finish
            for s in range(NBUF):
                uses = len(range(s, n_rounds, NBUF))
                if uses:
                    sync.wait_ge(d_sems[s], 16 * uses)

        @block.vector
        def _(vector):
            vector.wait_ge(in_sem, 16)
            for r, (b, g0, g) in enumerate(rounds):
                s = r % NBUF
                if r >= NBUF:
                    # slot's previous DMA (round r-NBUF) fully drained
                    vector.wait_ge(d_sems[s], 16 * (r // NBUF))
                last = None
                for k in range(g):
                    col = T + b * ntiles + g0 + k
                    last = nc.vector.tensor_scalar(
                        out=bufs[s][:, k * T : (k + 1) * T],
                        in0=sb[:, 0:T],
                        scalar1=sb[:, col : col + 1],
                        scalar2=None,
                        op0=mybir.AluOpType.is_equal,
                    )
                last.then_inc(c_sem, 1)

    return nc


def _token_ids(dur: np.ndarray, m_pad: int) -> np.ndarray:
    """tid[b, f] = index of the token whose frame interval contains f,
    or T (out of range -> all-zero output row) when no token covers f."""
    ends = np.cumsum(dur.astype(np.int64), axis=1)
    frames = np.arange(m_pad, dtype=np.int64)
    tid = np.empty((dur.shape[0], m_pad), dtype=np.float32)
    for b in range(dur.shape[0]):
        tid[b] = np.searchsorted(ends[b], frames, side="right")
    return tid


def _prepare(duration_predictor_output: np.ndarray, max_frames):
    """Host-side prep: token ids, per-core input maps, cached Bass graph."""
    dur = np.asarray(duration_predictor_output)
    m_frames = int(max_frames)
    b_loc = B // N_CORES
    ntiles = math.ceil(m_frames / P)
    m_pad = ntiles * P

    tid = _token_ids(dur, m_pad)  # [B, m_pad] float32
    iota_row = np.broadcast_to(np.arange(T, dtype=np.float32), (P, T))

    key = (m_frames, b_loc)
    nc = _nc_cache.get(key)
    if nc is None:
        nc = _build(m_frames, b_loc)
        _nc_cache[key] = nc

    in_maps = []
    for i in range(N_CORES):
        rows = tid[i * b_loc : (i + 1) * b_loc]              # [b_loc, m_pad]
        # partition p holds the contiguous frame span [p*ntiles,(p+1)*ntiles)
        tid_t = rows.reshape(b_loc, P, ntiles).transpose(1, 0, 2).reshape(P, -1)
        in_maps.append(
            {"tid": np.ascontiguousarray(np.concatenate([iota_row, tid_t], axis=1))}
        )
    return nc, in_maps


def kernel(duration_predictor_output: np.ndarray, max_frames) -> np.ndarray:
    dur = np.asarray(duration_predictor_output)
    m_frames = int(max_frames)
    if m_frames <= 0:
        return np.zeros((dur.shape[0], 0, dur.shape[1]), dtype=np.float32)

    nc, in_maps = _prepare(dur, m_frames)
    res = run_bass_kernel_spmd(nc, in_maps, core_ids=list(range(N_CORES)))
    full = np.concatenate([res.results[i]["out"] for i in range(N_CORES)], axis=0)
    return np.ascontiguousarray(full[:, :m_frames, :])


# revision 23
# speedup vs baseline: 1.0114x; 1.0114x over previous
"""Alignment generator (length regulator) on 8 TRN2 NeuronCores.

out[b, f, j] = 1.0  iff  starts[b,j] <= f < ends[b,j]  (ends = cumsum(dur))

Each output row out[b, f, :] is one-hot at token_id[b, f] =
searchsorted(ends[b], f, side='right') (or all-zero when no token covers
frame f). The host computes token_id from the tiny [32, 512] duration input;
each core then generates its 4-row slab of the ~256MB output with one DVE
tensor_scalar(is_equal) per [128, 512] tile (fp32 tensor_scalar runs in the
2x_2P perf mode) and streams it out in ~2MB HWDGE DMAs. The kernel is
write-bandwidth bound: per core ~32MB at the ~358 GB/s HBM-per-core limit
is ~90us, and measured NEFF time is ~94-105us.

Layout: partition p of row b covers the CONTIGUOUS frame span
[p*ntiles, (p+1)*ntiles), so every output DMA is perfectly linear in DRAM
(16KB+ contiguous per partition-descriptor instead of 2KB tile rows).

Raw Bass (no Tile): this walrus build only allows a single sync-wait per
compute/DMA instruction, so all synchronization is explicit standalone
wait_ge with a ring of NBUF buffers and one completion semaphore per buffer
slot (per-slot sems make "slot's previous DMA fully drained" provable from
a 16*m threshold; a single shared sem cannot distinguish engine skew).

Sharding: pure data parallelism, batch dim 32 -> 4 rows per core; no
collectives.
"""

import math
from contextlib import ExitStack

import numpy as np

import concourse.bass as bass
import concourse.mybir as mybir
from concourse.bass_utils import run_bass_kernel_spmd

N_CORES = 8
B = 32          # batch
T = 512         # tokens
P = 128         # SBUF partitions
GROUP = 8       # span steps per output DMA (8*128*512*4B = 2 MiB)
NBUF = 4        # output buffer ring slots

_nc_cache: dict[tuple[int, int], bass.Bass] = {}


def _build(m_frames: int, b_loc: int) -> bass.Bass:
    """Per-core Bass graph writing a [b_loc, m_pad, T] padded output slab."""
    ntiles = math.ceil(m_frames / P)
    m_pad = ntiles * P

    # rounds: (row, first_span_step, n_span_steps). Ramp the first row's
    # chunks (1,1,2,4,...) so the first output DMA is issued as soon as
    # possible after the input lands -- the DMA stream is the bottleneck
    # and every ns it starts earlier is a ns off the kernel.
    rounds = []
    for b in range(b_loc):
        g0 = 0
        for g in [1, 1, 2, 4] if b == 0 else []:
            if g0 + g > ntiles:
                break
            rounds.append((b, g0, g))
            g0 += g
        while g0 < ntiles:
            g = min(GROUP, ntiles - g0)
            rounds.append((b, g0, g))
            g0 += g
    n_rounds = len(rounds)

    nc = bass.Bass()
    # input column (b*ntiles + i) = token ids of frames {p*ntiles + i} of
    # row b; the iota row J (J[p,j] = j) is generated on-device by GpSimd
    # in parallel with this DMA, off the fill critical path
    tid = nc.declare_dram_parameter(
        "tid", [P, b_loc * ntiles], mybir.dt.float32, isOutput=False
    )
    out = nc.declare_dram_parameter(
        "out", [b_loc, m_pad, T], mybir.dt.float32, isOutput=True
    )

    with ExitStack() as ctx:
        sb = ctx.enter_context(
            nc.sbuf_tensor("sb", [P, b_loc * ntiles], mybir.dt.float32)
        )
        Jsb = ctx.enter_context(nc.sbuf_tensor("J", [P, T], mybir.dt.float32))
        bufs = [
            ctx.enter_context(
                nc.sbuf_tensor(f"buf{s}", [P, GROUP * T], mybir.dt.float32)
            )
            for s in range(NBUF)
        ]
        in_sem = ctx.enter_context(nc.semaphore("in_sem"))
        j_sem = ctx.enter_context(nc.semaphore("j_sem"))
        c_sem = ctx.enter_context(nc.semaphore("c_sem"))
        d_sems = [ctx.enter_context(nc.semaphore(f"d_sem{s}")) for s in range(NBUF)]
        block = ctx.enter_context(nc.Block())

        @block.gpsimd
        def _(gpsimd):
            # values 0..511 are exact in fp32
            gpsimd.iota(
                Jsb[:, :],
                pattern=[[1, T]],
                base=0,
                channel_multiplier=0,
                allow_small_or_imprecise_dtypes=True,
            ).then_inc(j_sem, 1)

        @block.sync
        def _(sync):
            sync.dma_start(out=sb[:, :], in_=tid[:, :]).then_inc(in_sem, 16)
            for r, (b, g0, g) in enumerate(rounds):
                sync.wait_ge(c_sem, r + 1)
                dview = out[b].rearrange("(p i) t -> p (i t)", p=P)[
                    :, g0 * T : (g0 + g) * T
                ]
                sync.dma_start(
                    out=dview, in_=bufs[r % NBUF][:, : g * T]
                ).then_inc(d_sems[r % NBUF], 16)
            # all output bytes landed before the NEFF may finish
            for s in range(NBUF):
                uses = len(range(s, n_rounds, NBUF))
                if uses:
                    sync.wait_ge(d_sems[s], 16 * uses)

        @block.vector
        def _(vector):
            vector.wait_ge(j_sem, 1)
            vector.wait_ge(in_sem, 16)
            for r, (b, g0, g) in enumerate(rounds):
                s = r % NBUF
                if r >= NBUF:
                    # slot's previous DMA (round r-NBUF) fully drained
                    vector.wait_ge(d_sems[s], 16 * (r // NBUF))
                last = None
                for k in range(g):
                    col = b * ntiles + g0 + k
                    last = nc.vector.tensor_scalar(
                        out=bufs[s][:, k * T : (k + 1) * T],
                        in0=Jsb[:, :],
                        scalar1=sb[:, col : col + 1],
                        scalar2=None,
                        op0=mybir.AluOpType.is_equal,
                    )
                last.then_inc(c_sem, 1)

    return nc


def _token_ids(dur: np.ndarray, m_pad: int) -> np.ndarray:
    """tid[b, f] = index of the token whose frame interval contains f,
    or T (out of range -> all-zero output row) when no token covers f."""
    ends = np.cumsum(dur.astype(np.int64), axis=1)
    frames = np.arange(m_pad, dtype=np.int64)
    tid = np.empty((dur.shape[0], m_pad), dtype=np.float32)
    for b in range(dur.shape[0]):
        tid[b] = np.searchsorted(ends[b], frames, side="right")
    return tid


def _prepare(duration_predictor_output: np.ndarray, max_frames):
    """Host-side prep: token ids, per-core input maps, cached Bass graph."""
    dur = np.asarray(duration_predictor_output)
    m_frames = int(max_frames)
    b_loc = B // N_CORES
    ntiles = math.ceil(m_frames / P)
    m_pad = ntiles * P

    tid = _token_ids(dur, m_pad)  # [B, m_pad] float32

    key = (m_frames, b_loc)
    nc = _nc_cache.get(key)
    if nc is None:
        nc = _build(m_frames, b_loc)
        _nc_cache[key] = nc

    in_maps = []
    for i in range(N_CORES):
        rows = tid[i * b_loc : (i + 1) * b_loc]              # [b_loc, m_pad]
        # partition p holds the contiguous frame span [p*ntiles,(p+1)*ntiles)
        tid_t = rows.reshape(b_loc, P, ntiles).transpose(1, 0, 2).reshape(P, -1)
        in_maps.append({"tid": np.ascontiguousarray(tid_t)})
    return nc, in_maps


def kernel(duration_predictor_output: np.ndarray, max_frames) -> np.ndarray:
    dur = np.asarray(duration_predictor_output)
    m_frames = int(max_frames)
    if m_frames <= 0:
        return np.zeros((dur.shape[0], 0, dur.shape[1]), dtype=np.float32)

    nc, in_maps = _prepare(dur, m_frames)
    res = run_bass_kernel_spmd(nc, in_maps, core_ids=list(range(N_CORES)))
    full = np.concatenate([res.results[i]["out"] for i in range(N_CORES)], axis=0)
    return np.ascontiguousarray(full[:, :m_frames, :])
